# revision 21
# baseline (speedup 1.0000x reference)
"""Trainium2 Bass kernel for CellPathwayPoolingAggregator (segment mean).

out[b, p] = (1/segment_sizes[p]) * sum_{k: segment_ids[k]==p} x[b, flat_indices[k]]

Strategy (8 cores, sharded by contiguous pathway ranges):
  - Host: transpose x -> xT (G, B) fp16 (one shared copy per core). Split the
    1000 pathways into 8 contiguous ranges (<=128 pathways each) with roughly
    equal index counts.
  - Device (per core): gpsimd.dma_gather pulls the core's ~K/8 indexed
    gene-rows (full 2048-batch rows, 4KB each -> full DMA bandwidth) from
    DRAM into SBUF laid out [k%128, k//128, b]. For each 128-row K-tile a PE
    matmul with the per-core one-hot matrix S (128 k-rows x 128 local
    pathways, stationary) streams the gathered rows (N=512 x 4) and
    accumulates pathway x batch sums into one 4-bank PSUM tile.
  - DVE scales rows by 1/segment_sizes (per-partition scalar), DMA stores
    the (128, 2048) f32 transposed output slice; host reassembles/transposes.

All data-dependence lives in the per-core idx/S tensors, so the single SPMD
program is uniform across cores (T_max tiles each, zero-padded).
"""

import sys

import numpy as np

_TRN_REPO = "/opt/trn_rl_repo"
if _TRN_REPO not in sys.path:
    sys.path.insert(0, _TRN_REPO)

import concourse.bass as bass  # noqa: F401  (AP helpers via bass)
import concourse.mybir as mybir
import concourse.tile as tile
from concourse import bacc
from concourse.bass_utils import run_bass_kernel_spmd

B, G, P = 2048, 10000, 1000
NCORES = 8
PC = 128          # max pathways per core (psum partition dim)
NB = B // 512     # matmul N-slices per K-tile (4 banks of 512 f32)
CH = 4            # gather-chunk size in 128-index tiles (<=1024 idxs per dma_gather)
NQ = 4            # SWDGE queues, round-robin per chunk


def _split_ranges(seg_sorted):
    """Contiguous pathway ranges, <=128 pathways each, ~equal index counts."""
    K = len(seg_sorted)
    cnt = np.bincount(seg_sorted, minlength=P)
    cum = np.concatenate([[0], np.cumsum(cnt)])  # cum[p] = #entries below p
    bounds = [0]
    for c in range(1, NCORES):
        target = K * c // NCORES
        b = int(np.searchsorted(cum, target))
        b = max(bounds[-1] + 1, min(b, bounds[-1] + PC))
        b = max(b, P - PC * (NCORES - c))  # leave room for remaining cores
        b = min(b, P - (NCORES - c))       # leave >=1 pathway per core
        bounds.append(b)
    bounds.append(P)
    return bounds


def _build_schedule(flat_indices, segment_ids):
    seg = np.asarray(segment_ids, dtype=np.int64)
    idx = np.asarray(flat_indices, dtype=np.int64)
    order = np.argsort(seg, kind="stable")
    seg = seg[order]
    idx = idx[order]

    bounds = _split_ranges(seg)
    cores = []
    for c in range(NCORES):
        lo_p, hi_p = bounds[c], bounds[c + 1]
        lo = np.searchsorted(seg, lo_p, side="left")
        hi = np.searchsorted(seg, hi_p, side="left")
        # Deduplicate gene rows within the core: each distinct gene is
        # gathered once; S accumulates per-(gene,pathway) counts (exact in
        # fp16 for the counts seen here).
        uidx, inv = np.unique(idx[lo:hi], return_inverse=True)
        cores.append((lo_p, hi_p, uidx, inv, seg[lo:hi] - lo_p))

    T = max(1, max((len(u) + 127) // 128 for _, _, u, _, _ in cores))
    Kpad = T * 128

    idx_sbs, s_sbs = [], []
    for lo_p, hi_p, uidx, inv, cols in cores:
        nu = len(uidx)
        idx_p = np.concatenate([uidx, np.zeros(Kpad - nu, np.int64)])
        S = np.zeros((Kpad, PC), np.float32)
        np.add.at(S, (inv, cols), 1.0)
        S = S.astype(np.float16)
        s_sbs.append(
            np.ascontiguousarray(
                S.reshape(T, 128, PC).transpose(1, 0, 2).reshape(128, T * PC)
            )
        )
        idx16 = idx_p.astype(np.int16)
        idx_sbs.append(
            np.ascontiguousarray(np.tile(idx16.reshape(Kpad // 16, 16).T, (8, 1)))
        )
    return bounds, idx_sbs, s_sbs, T


def _build_program(T):
    nc = bacc.Bacc(
        "TRN2",
        target_bir_lowering=False,
        debug=False,
        num_devices=NCORES,
        num_swdge_queues=NQ,
    )
    f16, f32, i16 = mybir.dt.float16, mybir.dt.float32, mybir.dt.int16

    xt_d = nc.dram_tensor("xt", [G, B], f16, kind="ExternalInput")
    idx_d = nc.dram_tensor("idx", [128, T * 8], i16, kind="ExternalInput")
    s_d = nc.dram_tensor("smat", [128, T * PC], f16, kind="ExternalInput")
    inv_d = nc.dram_tensor("invsz", [128, 1], f32, kind="ExternalInput")
    out_d = nc.dram_tensor("out", [PC, B], f32, kind="ExternalOutput")

    with tile.TileContext(nc) as tc:
        with (
            tc.tile_pool(name="const", bufs=1) as cpool,
            tc.tile_pool(name="gather", bufs=6) as gpool,
            tc.tile_pool(name="psum", bufs=1, space="PSUM") as ppool,
            tc.tile_pool(name="outp", bufs=1) as opool,
        ):
            idx_sb = cpool.tile([128, T * 8], i16, tag="idx")
            nc.sync.dma_start(idx_sb[:], idx_d.ap())
            s_sb = cpool.tile([128, T * PC], f16, tag="smat")
            nc.sync.dma_start(s_sb[:], s_d.ap())
            inv_sb = cpool.tile([128, 1], f32, tag="invsz")
            nc.sync.dma_start(inv_sb[:], inv_d.ap())

            ps = ppool.tile([128, B], f32, tag="ps", name="ps")

            # Ramp-up chunk sizes: small first chunks so PE gets data early
            # (the first gather sits behind the GpSimd library load), then
            # full CH-tile chunks (1024-idx dma_gather cap).
            chunks = []
            t_begin = 0
            for size in (1, 2):
                if t_begin < T:
                    n_t = min(size, T - t_begin)
                    chunks.append((t_begin, n_t))
                    t_begin += n_t
            while t_begin < T:
                n_t = min(CH, T - t_begin)
                chunks.append((t_begin, n_t))
                t_begin += n_t

            for c, (t_begin, n_t) in enumerate(chunks):
                gt = gpool.tile([128, CH, B], f16, tag="gt")
                n_idx = n_t * 128
                nc.gpsimd.dma_gather(
                    gt[:, 0:n_t, :],
                    xt_d.ap(),
                    idx_sb[:, t_begin * 8 : t_begin * 8 + n_t * 8],
                    num_idxs=n_idx,
                    num_idxs_reg=n_idx,
                    elem_size=B,
                    queue_num=c % NQ,
                )
                # Bank-major order: consecutive matmuls hit the same PSUM
                # bank, minimizing bank-switch micro-idles on PE.
                for n in range(NB):
                    for tt in range(t_begin, t_begin + n_t):
                        nc.tensor.matmul(
                            ps[:, n * 512 : (n + 1) * 512],
                            s_sb[:, tt * PC : (tt + 1) * PC],
                            gt[:, tt - t_begin, n * 512 : (n + 1) * 512],
                            # Every matmul writes the full (128, 512) bank
                            # region, so tile 0 with start=True both clears
                            # the bank's has_written bits and seeds the sums.
                            start=(tt == 0),
                            stop=(tt == T - 1),
                        )

            # Per-bank eviction + store so bank n ships while bank n+1's last
            # matmul is still draining.
            for n in range(NB):
                ot = opool.tile([128, 512], f32, tag=f"ot{n}", name=f"ot{n}")
                nc.vector.tensor_scalar_mul(
                    ot[:], ps[:, n * 512 : (n + 1) * 512], inv_sb[:]
                )
                nc.sync.dma_start(out_d.ap()[:, n * 512 : (n + 1) * 512], ot[:])
    return nc


def _prepare(gene_set_features, flat_indices, segment_ids, segment_sizes):
    bounds, idx_sbs, s_sbs, T = _build_schedule(flat_indices, segment_ids)
    nc = _build_program(T)
    nc.compile()

    x = np.asarray(gene_set_features, dtype=np.float32)
    xt16 = np.ascontiguousarray(x.T.astype(np.float16))  # (G, B)
    sizes = np.asarray(segment_sizes, dtype=np.float32)

    in_maps = []
    for c in range(NCORES):
        lo_p, hi_p = bounds[c], bounds[c + 1]
        inv = np.ones((128, 1), np.float32)
        inv[: hi_p - lo_p, 0] = 1.0 / sizes[lo_p:hi_p]
        in_maps.append(
            {"xt": xt16, "idx": idx_sbs[c], "smat": s_sbs[c], "invsz": inv}
        )
    return nc, in_maps, bounds


def kernel(gene_set_features, flat_indices, segment_ids, segment_sizes, _res_hook=None):
    nc, in_maps, bounds = _prepare(
        gene_set_features, flat_indices, segment_ids, segment_sizes
    )
    res = run_bass_kernel_spmd(nc, in_maps, list(range(NCORES)))
    if _res_hook is not None:
        _res_hook(res)
    outT = np.empty((P, B), np.float32)
    for c in range(NCORES):
        lo_p, hi_p = bounds[c], bounds[c + 1]
        outT[lo_p:hi_p] = np.asarray(res.results[c]["out"])[: hi_p - lo_p]
    return np.ascontiguousarray(outT.T)


# revision 22
# speedup vs baseline: 1.1537x; 1.1537x over previous
"""Trainium2 Bass kernel for CellPathwayPoolingAggregator (segment mean).

out[b, p] = (1/segment_sizes[p]) * sum_{k: segment_ids[k]==p} x[b, flat_indices[k]]

Strategy (8 cores, sharded by contiguous pathway ranges):
  - Host: transpose x -> xT (G, B) fp16 (one shared copy per core). Split the
    1000 pathways into 8 contiguous ranges (<=128 pathways each) with roughly
    equal index counts.
  - Device (per core): gpsimd.dma_gather pulls the core's ~K/8 indexed
    gene-rows (full 2048-batch rows, 4KB each -> full DMA bandwidth) from
    DRAM into SBUF laid out [k%128, k//128, b]. For each 128-row K-tile a PE
    matmul with the per-core one-hot matrix S (128 k-rows x 128 local
    pathways, stationary) streams the gathered rows (N=512 x 4) and
    accumulates pathway x batch sums into one 4-bank PSUM tile.
  - DVE scales rows by 1/segment_sizes (per-partition scalar), DMA stores
    the (128, 2048) f32 transposed output slice; host reassembles/transposes.

All data-dependence lives in the per-core idx/S tensors, so the single SPMD
program is uniform across cores (T_max tiles each, zero-padded).
"""

import sys

import numpy as np

_TRN_REPO = "/opt/trn_rl_repo"
if _TRN_REPO not in sys.path:
    sys.path.insert(0, _TRN_REPO)

import concourse.bass as bass  # noqa: F401  (AP helpers via bass)
import concourse.mybir as mybir
import concourse.tile as tile
from concourse import bacc
from concourse.bass_utils import run_bass_kernel_spmd

B, G, P = 2048, 10000, 1000
NCORES = 8
PC = 128          # max pathways per core (psum partition dim)
NB = B // 512     # matmul N-slices per K-tile (4 banks of 512 f32)
CH = 4            # gather-chunk size in 128-index tiles (<=1024 idxs per dma_gather)
NQ = 4            # SWDGE queues, round-robin per chunk


def _split_ranges(seg_sorted):
    """Contiguous pathway ranges, <=128 pathways each, ~equal index counts."""
    K = len(seg_sorted)
    cnt = np.bincount(seg_sorted, minlength=P)
    cum = np.concatenate([[0], np.cumsum(cnt)])  # cum[p] = #entries below p
    bounds = [0]
    for c in range(1, NCORES):
        target = K * c // NCORES
        b = int(np.searchsorted(cum, target))
        b = max(bounds[-1] + 1, min(b, bounds[-1] + PC))
        b = max(b, P - PC * (NCORES - c))  # leave room for remaining cores
        b = min(b, P - (NCORES - c))       # leave >=1 pathway per core
        bounds.append(b)
    bounds.append(P)
    return bounds


def _build_schedule(flat_indices, segment_ids):
    seg = np.asarray(segment_ids, dtype=np.int64)
    idx = np.asarray(flat_indices, dtype=np.int64)
    order = np.argsort(seg, kind="stable")
    seg = seg[order]
    idx = idx[order]

    bounds = _split_ranges(seg)
    cores = []
    for c in range(NCORES):
        lo_p, hi_p = bounds[c], bounds[c + 1]
        lo = np.searchsorted(seg, lo_p, side="left")
        hi = np.searchsorted(seg, hi_p, side="left")
        # Deduplicate gene rows within the core: each distinct gene is
        # gathered once; S accumulates per-(gene,pathway) counts (exact in
        # fp16 for the counts seen here).
        uidx, inv = np.unique(idx[lo:hi], return_inverse=True)
        cores.append((lo_p, hi_p, uidx, inv, seg[lo:hi] - lo_p))

    T = max(1, max((len(u) + 127) // 128 for _, _, u, _, _ in cores))
    Kpad = T * 128

    idx_sbs, s_sbs = [], []
    for lo_p, hi_p, uidx, inv, cols in cores:
        nu = len(uidx)
        idx_p = np.concatenate([uidx, np.zeros(Kpad - nu, np.int64)])
        S = np.zeros((Kpad, PC), np.float32)
        np.add.at(S, (inv, cols), 1.0)
        S = S.astype(np.float16)
        s_sbs.append(
            np.ascontiguousarray(
                S.reshape(T, 128, PC).transpose(1, 0, 2).reshape(128, T * PC)
            )
        )
        idx16 = idx_p.astype(np.int16)
        idx_sbs.append(
            np.ascontiguousarray(np.tile(idx16.reshape(Kpad // 16, 16).T, (8, 1)))
        )
    return bounds, idx_sbs, s_sbs, T


def _build_program(T):
    nc = bacc.Bacc(
        "TRN2",
        target_bir_lowering=False,
        debug=False,
        num_devices=NCORES,
        num_swdge_queues=NQ,
    )
    f16, f32, i16 = mybir.dt.float16, mybir.dt.float32, mybir.dt.int16

    xt_d = nc.dram_tensor("xt", [G, B], f16, kind="ExternalInput")
    idx_d = nc.dram_tensor("idx", [128, T * 8], i16, kind="ExternalInput")
    s_d = nc.dram_tensor("smat", [128, T * PC], f16, kind="ExternalInput")
    inv_d = nc.dram_tensor("invsz", [128, 1], f32, kind="ExternalInput")
    out_d = nc.dram_tensor("out", [PC, B], f32, kind="ExternalOutput")

    with tile.TileContext(nc) as tc:
        with (
            tc.tile_pool(name="const", bufs=1) as cpool,
            tc.tile_pool(name="gather", bufs=6) as gpool,
            tc.tile_pool(name="psum", bufs=1, space="PSUM") as ppool,
            tc.tile_pool(name="outp", bufs=1) as opool,
        ):
            idx_sb = cpool.tile([128, T * 8], i16, tag="idx")
            nc.sync.dma_start(idx_sb[:], idx_d.ap())
            s_sb = cpool.tile([128, T * PC], f16, tag="smat")
            nc.sync.dma_start(s_sb[:], s_d.ap())
            inv_sb = cpool.tile([128, 1], f32, tag="invsz")
            nc.sync.dma_start(inv_sb[:], inv_d.ap())

            ps = ppool.tile([128, B], f32, tag="ps", name="ps")

            # Ramp-up chunk sizes: small first chunks so PE gets data early
            # (the first gather sits behind the GpSimd library load), then
            # full CH-tile chunks (1024-idx dma_gather cap).
            chunks = []
            t_begin = 0
            for size in (1, 2):
                if t_begin < T:
                    n_t = min(size, T - t_begin)
                    chunks.append((t_begin, n_t))
                    t_begin += n_t
            while t_begin < T:
                n_t = min(CH, T - t_begin)
                chunks.append((t_begin, n_t))
                t_begin += n_t

            for c, (t_begin, n_t) in enumerate(chunks):
                gt = gpool.tile([128, CH, B], f16, tag="gt")
                n_idx = n_t * 128
                nc.gpsimd.dma_gather(
                    gt[:, 0:n_t, :],
                    xt_d.ap(),
                    idx_sb[:, t_begin * 8 : t_begin * 8 + n_t * 8],
                    num_idxs=n_idx,
                    num_idxs_reg=n_idx,
                    elem_size=B,
                    queue_num=c % NQ,
                )
                for tt in range(t_begin, t_begin + n_t):
                    for n in range(NB):
                        nc.tensor.matmul(
                            ps[:, n * 512 : (n + 1) * 512],
                            s_sb[:, tt * PC : (tt + 1) * PC],
                            gt[:, tt - t_begin, n * 512 : (n + 1) * 512],
                            # Every matmul writes the full (128, 512) bank
                            # region, so tile 0 with start=True both clears
                            # the bank's has_written bits and seeds the sums.
                            start=(tt == 0),
                            stop=(tt == T - 1),
                        )

            # Per-bank eviction + store so bank n ships while bank n+1's last
            # matmul is still draining.
            for n in range(NB):
                ot = opool.tile([128, 512], f32, tag=f"ot{n}", name=f"ot{n}")
                nc.vector.tensor_scalar_mul(
                    ot[:], ps[:, n * 512 : (n + 1) * 512], inv_sb[:]
                )
                nc.sync.dma_start(out_d.ap()[:, n * 512 : (n + 1) * 512], ot[:])
    return nc


def _prepare(gene_set_features, flat_indices, segment_ids, segment_sizes):
    bounds, idx_sbs, s_sbs, T = _build_schedule(flat_indices, segment_ids)
    nc = _build_program(T)
    nc.compile()

    x = np.asarray(gene_set_features, dtype=np.float32)
    xt16 = np.ascontiguousarray(x.T.astype(np.float16))  # (G, B)
    sizes = np.asarray(segment_sizes, dtype=np.float32)

    in_maps = []
    for c in range(NCORES):
        lo_p, hi_p = bounds[c], bounds[c + 1]
        inv = np.ones((128, 1), np.float32)
        inv[: hi_p - lo_p, 0] = 1.0 / sizes[lo_p:hi_p]
        in_maps.append(
            {"xt": xt16, "idx": idx_sbs[c], "smat": s_sbs[c], "invsz": inv}
        )
    return nc, in_maps, bounds


def kernel(gene_set_features, flat_indices, segment_ids, segment_sizes, _res_hook=None):
    nc, in_maps, bounds = _prepare(
        gene_set_features, flat_indices, segment_ids, segment_sizes
    )
    res = run_bass_kernel_spmd(nc, in_maps, list(range(NCORES)))
    if _res_hook is not None:
        _res_hook(res)
    outT = np.empty((P, B), np.float32)
    for c in range(NCORES):
        lo_p, hi_p = bounds[c], bounds[c + 1]
        outT[lo_p:hi_p] = np.asarray(res.results[c]["out"])[: hi_p - lo_p]
    return np.ascontiguousarray(outT.T)


# revision 24
# speedup vs baseline: 1.1916x; 1.0328x over previous
"""Trainium2 Bass kernel for CellPathwayPoolingAggregator (segment mean).

out[b, p] = (1/segment_sizes[p]) * sum_{k: segment_ids[k]==p} x[b, flat_indices[k]]

Strategy (8 cores, sharded by contiguous pathway ranges):
  - Host: transpose x -> xT (G, B) fp16 (one shared copy per core). Split the
    1000 pathways into 8 contiguous ranges (<=128 pathways each) balancing
    per-core unique-gene counts; dedupe each core's gene rows (S carries
    counts).
  - Device (per core): one gpsimd.indirect_dma_start per 128-gene K-tile
    pulls the indexed gene-rows (full 2048-batch rows, 4KB each -> full DMA
    bandwidth) from DRAM into SBUF, row k -> partition k. A PE matmul with
    the per-core count matrix S (128 k-rows x 128 local pathways, stationary)
    streams the gathered rows (N=512 x 4) and accumulates pathway x batch
    sums into one 4-bank PSUM tile.
  - DVE scales rows by 1/segment_sizes (per-partition scalar), DMA stores
    the (128, 2048) f32 transposed output slice; host reassembles/transposes.

All data-dependence lives in the per-core idx/S tensors, so the single SPMD
program is uniform across cores (T_max tiles each, zero-padded).
"""

import sys

import numpy as np

_TRN_REPO = "/opt/trn_rl_repo"
if _TRN_REPO not in sys.path:
    sys.path.insert(0, _TRN_REPO)

import concourse.bass as bass  # noqa: F401  (AP helpers via bass)
import concourse.mybir as mybir
import concourse.tile as tile
from concourse import bacc
from concourse.bass_utils import run_bass_kernel_spmd

B, G, P = 2048, 10000, 1000
NCORES = 8
PC = 128          # max pathways per core (psum partition dim)
NB = B // 512     # matmul N-slices per K-tile (4 banks of 512 f32)
CH = 4            # gather-chunk size in 128-index tiles (<=1024 idxs per dma_gather)
NQ = 4            # SWDGE queues, round-robin per chunk


def _split_ranges(seg_sorted, idx_sorted):
    """Contiguous pathway ranges, <=128 pathways each, minimizing the max
    per-core count of UNIQUE genes (which sets T and hence DMA/PE work)."""
    seg_starts = np.searchsorted(seg_sorted, np.arange(P + 1), side="left")

    def feasible(U):
        bounds = [0]
        for c in range(NCORES):
            lo_p = bounds[-1]
            if lo_p >= P:
                return None
            # widest hi_p (<= lo_p+PC) whose unique-gene count <= U
            best = lo_p + 1
            hi_cap = min(P, lo_p + PC)
            lo_e = seg_starts[lo_p]
            for hi_p in range(lo_p + 1, hi_cap + 1):
                nu = len(np.unique(idx_sorted[lo_e : seg_starts[hi_p]]))
                if nu <= U:
                    best = hi_p
                else:
                    break
            bounds.append(best)
        return bounds if bounds[-1] >= P else None

    # binary search the smallest feasible U (in tile units for speed)
    lo_t, hi_t = 1, (len(idx_sorted) + 127) // 128 + 1
    best_bounds = None
    while lo_t <= hi_t:
        mid = (lo_t + hi_t) // 2
        b = feasible(mid * 128)
        if b is not None:
            best_bounds = b
            hi_t = mid - 1
        else:
            lo_t = mid + 1
    if best_bounds is None:
        best_bounds = list(
            np.minimum(np.arange(NCORES + 1) * ((P + NCORES - 1) // NCORES), P)
        )
    best_bounds[-1] = P
    return best_bounds


def _build_schedule(flat_indices, segment_ids):
    seg = np.asarray(segment_ids, dtype=np.int64)
    idx = np.asarray(flat_indices, dtype=np.int64)
    order = np.argsort(seg, kind="stable")
    seg = seg[order]
    idx = idx[order]

    bounds = _split_ranges(seg, idx)
    cores = []
    for c in range(NCORES):
        lo_p, hi_p = bounds[c], bounds[c + 1]
        lo = np.searchsorted(seg, lo_p, side="left")
        hi = np.searchsorted(seg, hi_p, side="left")
        # Deduplicate gene rows within the core: each distinct gene is
        # gathered once; S accumulates per-(gene,pathway) counts (exact in
        # fp16 for the counts seen here).
        uidx, inv = np.unique(idx[lo:hi], return_inverse=True)
        cores.append((lo_p, hi_p, uidx, inv, seg[lo:hi] - lo_p))

    T = max(1, max((len(u) + 127) // 128 for _, _, u, _, _ in cores))
    Kpad = T * 128

    idx_sbs, s_sbs = [], []
    for lo_p, hi_p, uidx, inv, cols in cores:
        nu = len(uidx)
        idx_p = np.concatenate([uidx, np.zeros(Kpad - nu, np.int64)])
        S = np.zeros((Kpad, PC), np.float32)
        np.add.at(S, (inv, cols), 1.0)
        S = S.astype(np.float16)
        s_sbs.append(
            np.ascontiguousarray(
                S.reshape(T, 128, PC).transpose(1, 0, 2).reshape(128, T * PC)
            )
        )
        idx_sbs.append(
            np.ascontiguousarray(idx_p.astype(np.int32).reshape(T, 128).T)
        )
    return bounds, idx_sbs, s_sbs, T


def _build_program(T):
    nc = bacc.Bacc(
        "TRN2",
        target_bir_lowering=False,
        debug=False,
        num_devices=NCORES,
        num_swdge_queues=1,
    )
    f16, f32, i32 = mybir.dt.float16, mybir.dt.float32, mybir.dt.int32

    xt_d = nc.dram_tensor("xt", [G, B], f16, kind="ExternalInput")
    idx_d = nc.dram_tensor("idx", [128, T], i32, kind="ExternalInput")
    s_d = nc.dram_tensor("smat", [128, T * PC], f16, kind="ExternalInput")
    inv_d = nc.dram_tensor("invsz", [128, 1], f32, kind="ExternalInput")
    out_d = nc.dram_tensor("out", [PC, B], f32, kind="ExternalOutput")

    with tile.TileContext(nc) as tc:
        with (
            tc.tile_pool(name="const", bufs=1) as cpool,
            tc.tile_pool(name="gather", bufs=8) as gpool,
            tc.tile_pool(name="psum", bufs=1, space="PSUM") as ppool,
            tc.tile_pool(name="outp", bufs=1) as opool,
        ):
            idx_sb = cpool.tile([128, T], i32, tag="idx")
            nc.sync.dma_start(idx_sb[:], idx_d.ap())
            s_sb = cpool.tile([128, T * PC], f16, tag="smat")
            nc.sync.dma_start(s_sb[:], s_d.ap())
            inv_sb = cpool.tile([128, 1], f32, tag="invsz")
            nc.sync.dma_start(inv_sb[:], inv_d.ap())

            ps = ppool.tile([128, B], f32, tag="ps", name="ps")

            for tt in range(T):
                gt = gpool.tile([128, B], f16, tag="gt")
                nc.gpsimd.indirect_dma_start(
                    out=gt[:],
                    out_offset=None,
                    in_=xt_d.ap(),
                    in_offset=bass.IndirectOffsetOnAxis(
                        ap=idx_sb[:, tt : tt + 1], axis=0
                    ),
                )
                for n in range(NB):
                    nc.tensor.matmul(
                        ps[:, n * 512 : (n + 1) * 512],
                        s_sb[:, tt * PC : (tt + 1) * PC],
                        gt[:, n * 512 : (n + 1) * 512],
                        # Every matmul writes the full (128, 512) bank
                        # region, so tile 0 with start=True both clears
                        # the bank's has_written bits and seeds the sums.
                        start=(tt == 0),
                        stop=(tt == T - 1),
                    )

            # Per-bank eviction + store so bank n ships while bank n+1's last
            # matmul is still draining.
            for n in range(NB):
                ot = opool.tile([128, 512], f32, tag=f"ot{n}", name=f"ot{n}")
                nc.vector.tensor_scalar_mul(
                    ot[:], ps[:, n * 512 : (n + 1) * 512], inv_sb[:]
                )
                nc.sync.dma_start(out_d.ap()[:, n * 512 : (n + 1) * 512], ot[:])
    return nc


def _prepare(gene_set_features, flat_indices, segment_ids, segment_sizes):
    bounds, idx_sbs, s_sbs, T = _build_schedule(flat_indices, segment_ids)
    nc = _build_program(T)
    nc.compile()

    x = np.asarray(gene_set_features, dtype=np.float32)
    xt16 = np.ascontiguousarray(x.T.astype(np.float16))  # (G, B)
    sizes = np.asarray(segment_sizes, dtype=np.float32)

    in_maps = []
    for c in range(NCORES):
        lo_p, hi_p = bounds[c], bounds[c + 1]
        inv = np.ones((128, 1), np.float32)
        inv[: hi_p - lo_p, 0] = 1.0 / sizes[lo_p:hi_p]
        in_maps.append(
            {"xt": xt16, "idx": idx_sbs[c], "smat": s_sbs[c], "invsz": inv}
        )
    return nc, in_maps, bounds


def kernel(gene_set_features, flat_indices, segment_ids, segment_sizes, _res_hook=None):
    nc, in_maps, bounds = _prepare(
        gene_set_features, flat_indices, segment_ids, segment_sizes
    )
    res = run_bass_kernel_spmd(nc, in_maps, list(range(NCORES)))
    if _res_hook is not None:
        _res_hook(res)
    outT = np.empty((P, B), np.float32)
    for c in range(NCORES):
        lo_p, hi_p = bounds[c], bounds[c + 1]
        outT[lo_p:hi_p] = np.asarray(res.results[c]["out"])[: hi_p - lo_p]
    return np.ascontiguousarray(outT.T)
